# revision 2
# baseline (speedup 1.0000x reference)
"""Multi-head causal self-attention (B=256, T=256, C=384, H=6, D=64) on 8
Trainium2 NeuronCores, data-parallel over batch (32 batches/core).

v3 over the baseline:
  - P@V and softmax denominators use fp8e4 DoubleRow matmuls (K=256 in one
    pass, 0.5 cycles/row): exp writes st in fp8 directly.  V is carried as
    v8 + v8r (fp8 value + fp8 residual) and P@V accumulates two DR matmuls,
    recovering ~bf16 V precision.  Head pairs pack along the free dim of one
    PSUM bank [64, 2, 256].
  - The denominator reciprocal is exp(-ln d) on the scalar engine: Ln and
    Exp share one activation table (natural_log_exp_and_others), so the
    engine never reloads tables (native DVE reciprocal measures ~8 cyc/elem
    and custom DVE ops don't lower on this walrus).
  - Causal mask + masked-block zeroing is ONE bitwise AND per head pair on
    the DVE over an int16 view of the fp8 st tile (byte-pair masks encode
    the tri/1/0/tri blocks exactly; 2-byte SBUF operands take the fast path).
  - Normalize writes odd heads partition-shifted (64:128) so the output
    projection keeps K=128 contraction.
"""

import numpy as np

import concourse.bass as bass
import concourse.tile as tile
from concourse import mybir
from concourse.bass_utils import run_bass_kernel_spmd

P = 128
B, T, C = 256, 256, 384
H, D = 6, 64
NCORES = 8
BL = B // NCORES  # 32 batches per core
G = 4  # batch group for Q/K projection weight reuse
F32 = mybir.dt.float32
F32R = mybir.dt.float32r
BF16 = mybir.dt.bfloat16
FP8 = mybir.dt.float8e4
I16 = mybir.dt.int16
DR = mybir.MatmulPerfMode.DoubleRow


def _split_drain_waits(nc, cap=1):
    """This container's walrus rejects instructions carrying more than one
    sync wait ("Too many sync wait commands"); hoist extras onto no-ops
    inserted before (same engine => executed in order)."""
    n_new = 0
    for f in nc.m.functions:
        for bb in f.blocks:
            il = bb.instructions
            out = []
            changed = False
            for inst in list(il):
                si = getattr(inst, "sync_info", None)
                if si is not None and len(si.on_wait) > cap:
                    waits = list(si.on_wait)
                    extra, keep = waits[:-cap], waits[-cap:]
                    for i in range(0, len(extra), cap):
                        nop = mybir.InstNoOp(
                            name=f"I-waitsplit-{n_new}",
                            sync_info=mybir.SyncInfo(
                                on_wait=extra[i : i + cap], on_update=[]
                            ),
                            bass_nofuse=True,
                            engine=inst.engine,
                        )
                        n_new += 1
                        out.append(nop)
                    si.on_wait = keep
                    changed = True
                out.append(inst)
            if changed:
                il.clear()
                il.extend(out)
    return n_new


def build_module(split_waits=True):
    nc = bass.Bass("TRN2", target_bir_lowering=False, debug=False)

    xt_d = nc.dram_tensor("xt", [C, BL, T], BF16, kind="ExternalInput").ap()
    wq_d = nc.dram_tensor("wq", [C, C], BF16, kind="ExternalInput").ap()
    wk_d = nc.dram_tensor("wk", [C, C], BF16, kind="ExternalInput").ap()
    wv_d = nc.dram_tensor("wv", [C, C], BF16, kind="ExternalInput").ap()
    wp_d = nc.dram_tensor("wp", [C, C], BF16, kind="ExternalInput").ap()
    bp_d = nc.dram_tensor("bp", [C], F32R, kind="ExternalInput").ap()
    mask_d = nc.dram_tensor("mask", [P, 2, 2, T // 2], I16, kind="ExternalInput").ap()
    mask8_d = nc.dram_tensor("mask8", [P, 2, 2, T], FP8, kind="ExternalInput").ap()
    y_d = nc.dram_tensor("y", [BL, T, C], F32, kind="ExternalOutput").ap()

    with tile.TileContext(nc) as tc:
        with (
            tc.tile_pool(name="consts", bufs=1) as consts,
            tc.tile_pool(name="xg", bufs=2) as xg_pool,
            tc.tile_pool(name="qt", bufs=4) as qt_pool,
            tc.tile_pool(name="kt", bufs=4) as kt_pool,
            tc.tile_pool(name="vsb", bufs=10) as v_pool,
            tc.tile_pool(name="sts", bufs=8) as sts_pool,
            tc.tile_pool(name="rbs", bufs=4) as rb_pool,
            tc.tile_pool(name="ot", bufs=3) as ot_pool,
            tc.tile_pool(name="ysb", bufs=4) as y_pool,
            tc.tile_pool(name="psbig", bufs=3, space="PSUM") as ps_big,
            tc.tile_pool(name="ps384", bufs=2, space="PSUM") as ps_384,
            tc.tile_pool(name="psorb", bufs=3, space="PSUM") as ps_orb,
        ):
            # ---- constants ----
            wq_sb = consts.tile([P, 3, C], BF16)
            wk_sb = consts.tile([P, 3, C], BF16)
            wv_sb = consts.tile([P, 3, C], BF16)
            wp_sb = consts.tile([P, 3, C], BF16)
            for w_sb, w_d in ((wq_sb, wq_d), (wk_sb, wk_d), (wv_sb, wv_d), (wp_sb, wp_d)):
                nc.sync.dma_start(w_sb[:], w_d.rearrange("(co ci) e -> ci co e", ci=P))
            # partition-replication via rank-1 matmul (ones (x) row): step-0
            # partition-broadcast DMAs produce garbage on hardware.
            ones_row = consts.tile([1, P], F32)
            nc.vector.memset(ones_row[:], 1.0)
            ones_row_r = consts.tile([1, P], F32R)
            nc.scalar.activation(
                ones_row_r[:], ones_row[:], mybir.ActivationFunctionType.Copy
            )
            bp_row = consts.tile([1, C], F32R)
            nc.sync.dma_start(bp_row[:], bp_d[None, :])
            bp_sb = consts.tile([P, C], F32)
            mask_sb = consts.tile([P, 2, 2, T // 2], I16)
            mask8_sb = consts.tile([P, 2, 2, T], FP8)
            nc.sync.dma_start(mask8_sb[:], mask8_d[:])
            nc.sync.dma_start(mask_sb[:], mask_d[:])
            bp_ps = ps_384.tile([P, C], F32, tag="ps384")
            nc.tensor.matmul(bp_ps[:], ones_row_r[0:1, :], bp_row[0:1, :], start=True, stop=True)
            nc.vector.tensor_copy(bp_sb[:], bp_ps[:])

            ones_v = consts.tile([P, 2, 64], FP8)
            nc.vector.memset(ones_v[:], 1.0)

            xt_r = xt_d.rearrange("(co ci) b t -> ci co b t", ci=P)

            def emit_tail(b, st8s, v8, v8r):
                """P@V + denominators (fp8 DR) + normalize + O-proj + y for
                batch b.  Emitted one batch late so the next batch's score
                matmuls keep the PE busy through this tail's ACT/DVE chain."""
                ot = ot_pool.tile([P, 3, T], BF16)
                for hp in range(3):
                    st8 = st8s[hp]
                    o_ps = ps_orb.tile([64, 2, T], F32, tag="psorb")
                    rb_ps = ps_orb.tile([64, 2, T], F32, tag="psorb")
                    for hidx in range(2):
                        h = 2 * hp + hidx
                        nc.tensor.matmul(
                            o_ps[:, hidx, :],
                            v8[:, :, h, :],
                            st8[:, hidx, :, :],
                            start=True, stop=False, perf_mode=DR,
                        )
                        nc.tensor.matmul(
                            o_ps[:, hidx, :],
                            v8r[:, :, h, :],
                            st8[:, hidx, :, :],
                            start=False, stop=True, perf_mode=DR,
                        )
                        nc.tensor.matmul(
                            rb_ps[:, hidx, :],
                            ones_v[:],
                            st8[:, hidx, :, :],
                            start=True, stop=True, perf_mode=DR,
                        )
                    # 1/d = exp(-ln d): both functions live in the
                    # natural_log_exp_and_others table -> no table loads
                    lnb = rb_pool.tile([64, 2, T], F32, tag="lnb")
                    nc.scalar.activation(
                        lnb[:].rearrange("p a b -> p (a b)"),
                        rb_ps[:].rearrange("p a b -> p (a b)"),
                        mybir.ActivationFunctionType.Ln,
                    )
                    rb = rb_pool.tile([64, 2, T], BF16, tag="rb")
                    nc.scalar.activation(
                        rb[:].rearrange("p a b -> p (a b)"),
                        lnb[:].rearrange("p a b -> p (a b)"),
                        mybir.ActivationFunctionType.Exp,
                        scale=-1.0,
                    )
                    # even head -> ot partitions 0:64, odd -> 64:128
                    # (DVE writes support a shifted output base partition)
                    nc.vector.tensor_mul(
                        ot[0:64, hp, :], o_ps[:, 0, :], rb[:, 0, :]
                    )
                    nc.vector.tensor_mul(
                        ot[64:128, hp, :], o_ps[:, 1, :], rb[:, 1, :]
                    )

                for tt in range(2):
                    ps = ps_384.tile([P, C], F32, tag="ps384")
                    for co in range(3):
                        nc.tensor.matmul(
                            ps[:],
                            (ot[:, co, tt * P : (tt + 1) * P]),
                            (wp_sb[:, co, :]),
                            start=(co == 0),
                            stop=(co == 2),
                        )
                    y_sb = y_pool.tile([P, C], F32)
                    nc.vector.tensor_add(y_sb[:], ps[:], bp_sb[:])
                    nc.sync.dma_start(y_d[b, tt * P : (tt + 1) * P, :], y_sb[:])

            pending = None
            for g in range(BL // G):
                # ---- load x group [128, 3, G, T] ----
                xg = xg_pool.tile([P, 3, G, T], BF16)
                nc.sync.dma_start(xg[:], xt_r[:, :, g * G : (g + 1) * G, :])

                # ---- Q/K projections for the group (weights stationary) ----
                qt2s, kt2s = [], []
                for w_sb, dst_list in ((wq_sb, qt2s), (wk_sb, kt2s)):
                    pool = qt_pool if w_sb is wq_sb else kt_pool
                    tg = "qtb" if w_sb is wq_sb else "ktb"
                    for bp2 in range(G // 2):
                        dst_list.append(
                            pool.tile(
                                [P, 3, 2, T], BF16, tag=tg, name=f"{tg}_{g}_{bp2}"
                            )
                        )
                    for eo in range(3):
                        for bp2 in range(G // 2):
                            ps = ps_big.tile([P, 512], F32, tag="psbig")
                            rhs = xg[:, :, 2 * bp2 : 2 * bp2 + 2, :].rearrange(
                                "p c b t -> p c (b t)"
                            )
                            for co in range(3):
                                nc.tensor.matmul(
                                    ps[:],
                                    (w_sb[:, co, eo * P : (eo + 1) * P]),
                                    (rhs[:, co, :]),
                                    start=(co == 0),
                                    stop=(co == 2),
                                )
                            dst_ap = dst_list[bp2][:, eo, :, :].rearrange(
                                "p b t -> p (b t)"
                            )
                            nc.vector.tensor_copy(dst_ap, ps[:])

                # ---- V projections for the whole group (keeps the PE busy
                # through a long ramp and frees ps_384 for the O-proj) ----
                v8s, v8rs = [], []
                for lb in range(G):
                    v8 = v_pool.tile([P, 2, H, D], FP8, tag="v8")
                    v8r = v_pool.tile([P, 2, H, D], FP8, tag="v8r")
                    for tt in range(2):
                        ps = ps_384.tile([P, C], F32, tag="ps384")
                        for co in range(3):
                            nc.tensor.matmul(
                                ps[:],
                                (xg[:, co, lb, tt * P : (tt + 1) * P]),
                                (wv_sb[:, co, :]),
                                start=(co == 0),
                                stop=(co == 2),
                            )
                        nc.vector.tensor_copy(
                            v8[:, tt, :, :].rearrange("p h d -> p (h d)"), ps[:]
                        )
                        nc.vector.tensor_sub(
                            v8r[:, tt, :, :].rearrange("p h d -> p (h d)"),
                            ps[:],
                            v8[:, tt, :, :].rearrange("p h d -> p (h d)"),
                        )
                    v8s.append(v8)
                    v8rs.append(v8r)

                for lb in range(G):
                    b = g * G + lb
                    qt = qt2s[lb // 2][:, :, lb % 2, :]
                    kt = kt2s[lb // 2][:, :, lb % 2, :]

                    # ---- scores (transposed) + exp(fp8) + mask, per pair ----
                    # causal split: the jt1 keys block only matters for the
                    # upper half of queries; (jt1, q<128) holds stale psum
                    # which the mask zeroes (bitwise AND is NaN-safe).
                    st8s = []
                    for hp in range(3):
                        st8 = sts_pool.tile([P, 2, 2, T], FP8, tag="stp")
                        for hidx in range(2):
                            h = 2 * hp + hidx
                            co, half = h // 2, h % 2
                            st_ps = ps_big.tile([P, 2, T], F32, tag="psbig")
                            nc.tensor.matmul(
                                st_ps[:, 0, :],
                                (kt[64 * half : 64 * half + 64, co, 0:P]),
                                (qt[64 * half : 64 * half + 64, co, :]),
                                start=True,
                                stop=True,
                            )
                            nc.tensor.matmul(
                                st_ps[:, 1, P:T],
                                (kt[64 * half : 64 * half + 64, co, P:T]),
                                (qt[64 * half : 64 * half + 64, co, P:T]),
                                start=True,
                                stop=True,
                            )
                            nc.scalar.activation(
                                st8[:, hidx, :, :],
                                st_ps[:],
                                mybir.ActivationFunctionType.Exp,
                            )
                        # one int16-view bitwise AND applies the triangle to
                        # the diagonal blocks AND zeros the (jt1, q<128) block;
                        # the middle pair rides the otherwise-idle gpsimd as a
                        # plain fp8 multiply (no bitwise ops on Pool; stale
                        # values there are old finite exps, so x*0 == 0)
                        if hp == 1:
                            nc.gpsimd.tensor_mul(
                                st8[:].rearrange("p h a b -> p (h a b)"),
                                st8[:].rearrange("p h a b -> p (h a b)"),
                                mask8_sb[:].rearrange("p h a b -> p (h a b)"),
                            )
                        else:
                            st_i16 = st8[:].rearrange("p h a b -> p (h a b)").bitcast(I16)
                            nc.vector.tensor_tensor(
                                st_i16,
                                st_i16,
                                mask_sb[:].rearrange("p h a b -> p (h a b)"),
                                mybir.AluOpType.bitwise_and,
                            )
                        st8s.append(st8)

                    if pending is not None:
                        emit_tail(*pending)
                    pending = (b, st8s, v8s[lb], v8rs[lb])

            emit_tail(*pending)

    if split_waits:
        _split_drain_waits(nc)
    return nc


_NC = None


def _get_nc():
    global _NC
    if _NC is None:
        _NC = build_module()
    return _NC


def make_mask():
    # [128, 2(head), 2(jt), 128] int16 byte-pair visibility masks over the
    # fp8 st tile: key jt*128+p visible to query q when jt*128+p <= q.
    # Little-endian byte pair (q=2m, q=2m+1) -> 0xFF00 keeps q=2m+1 only.
    j = np.arange(P)[:, None]
    q = np.arange(T)[None, :]
    keep = np.empty((P, 2, T), bool)
    keep[:, 0, :] = j <= q          # keys 0:128
    keep[:, 1, :] = j + P <= q      # keys 128:256
    lo = keep[:, :, 0::2].astype(np.uint16) * 0x00FF
    hi = keep[:, :, 1::2].astype(np.uint16) * 0xFF00
    m = (lo | hi).view(np.int16)    # [P, 2(jt), 128]
    return np.broadcast_to(m[:, None, :, :], (P, 2, 2, T // 2)).copy()


def make_mask8():
    # same visibility as make_mask but as fp8 0/1 for the gpsimd multiply
    import ml_dtypes

    j = np.arange(P)[:, None]
    q = np.arange(T)[None, :]
    m = np.empty((P, 2, T), np.float32)
    m[:, 0, :] = j <= q
    m[:, 1, :] = j + P <= q
    m8 = m.astype(ml_dtypes.float8_e4m3)
    return np.broadcast_to(m8[:, None, :, :], (P, 2, 2, T)).copy()


def prepare_in_maps(x, Wk, Wq, Wv, Wp, bp):
    import ml_dtypes

    bf16 = ml_dtypes.bfloat16
    xt = np.ascontiguousarray(
        np.asarray(x, dtype=np.float32).transpose(2, 0, 1).astype(bf16)
    )
    # 1/sqrt(D) folded into Wq (exact exponent shift in bf16)
    wq = np.ascontiguousarray((np.asarray(Wq, dtype=np.float32).T * 0.125).astype(bf16))
    wk = np.ascontiguousarray(np.asarray(Wk, dtype=np.float32).T.astype(bf16))
    wv = np.ascontiguousarray(np.asarray(Wv, dtype=np.float32).T.astype(bf16))
    wp = np.ascontiguousarray(np.asarray(Wp, dtype=np.float32).T.astype(bf16))
    bp = np.asarray(bp, dtype=np.float32)
    mask = make_mask()
    mask8 = make_mask8()
    in_maps = []
    for c in range(NCORES):
        in_maps.append(
            {
                "xt": np.ascontiguousarray(xt[:, c * BL : (c + 1) * BL, :]),
                "wq": wq,
                "wk": wk,
                "wv": wv,
                "wp": wp,
                "bp": bp,
                "mask": mask,
                "mask8": mask8,
            }
        )
    return in_maps


def kernel(x, Wk, Wq, Wv, Wp, bp):
    nc = _get_nc()
    in_maps = prepare_in_maps(x, Wk, Wq, Wv, Wp, bp)
    res = run_bass_kernel_spmd(nc, in_maps, list(range(NCORES)))
    return np.concatenate([r["y"] for r in res.results], axis=0)


# revision 3
# speedup vs baseline: 1.0526x; 1.0526x over previous
"""Multi-head causal self-attention (B=256, T=256, C=384, H=6, D=64) on 8
Trainium2 NeuronCores, data-parallel over batch (32 batches/core).

v3 over the baseline:
  - P@V and softmax denominators use fp8e4 DoubleRow matmuls (K=256 in one
    pass, 0.5 cycles/row): exp writes st in fp8 directly.  V is carried as
    v8 + v8r (fp8 value + fp8 residual) and P@V accumulates two DR matmuls,
    recovering ~bf16 V precision.  Head pairs pack along the free dim of one
    PSUM bank [64, 2, 256].
  - The denominator reciprocal is exp(-ln d) on the scalar engine: Ln and
    Exp share one activation table (natural_log_exp_and_others), so the
    engine never reloads tables (native DVE reciprocal measures ~8 cyc/elem
    and custom DVE ops don't lower on this walrus).
  - Causal mask + masked-block zeroing is ONE bitwise AND per head pair on
    the DVE over an int16 view of the fp8 st tile (byte-pair masks encode
    the tri/1/0/tri blocks exactly; 2-byte SBUF operands take the fast path).
  - Normalize writes odd heads partition-shifted (64:128) so the output
    projection keeps K=128 contraction.
"""

import numpy as np

import concourse.bass as bass
import concourse.tile as tile
from concourse import mybir
from concourse.bass_utils import run_bass_kernel_spmd

P = 128
B, T, C = 256, 256, 384
H, D = 6, 64
NCORES = 8
BL = B // NCORES  # 32 batches per core
G = 4  # batch group for Q/K projection weight reuse
F32 = mybir.dt.float32
F32R = mybir.dt.float32r
BF16 = mybir.dt.bfloat16
FP8 = mybir.dt.float8e4
I16 = mybir.dt.int16
DR = mybir.MatmulPerfMode.DoubleRow


def _split_drain_waits(nc, cap=1):
    """This container's walrus rejects instructions carrying more than one
    sync wait ("Too many sync wait commands"); hoist extras onto no-ops
    inserted before (same engine => executed in order)."""
    n_new = 0
    for f in nc.m.functions:
        for bb in f.blocks:
            il = bb.instructions
            out = []
            changed = False
            for inst in list(il):
                si = getattr(inst, "sync_info", None)
                if si is not None and len(si.on_wait) > cap:
                    waits = list(si.on_wait)
                    extra, keep = waits[:-cap], waits[-cap:]
                    for i in range(0, len(extra), cap):
                        nop = mybir.InstNoOp(
                            name=f"I-waitsplit-{n_new}",
                            sync_info=mybir.SyncInfo(
                                on_wait=extra[i : i + cap], on_update=[]
                            ),
                            bass_nofuse=True,
                            engine=inst.engine,
                        )
                        n_new += 1
                        out.append(nop)
                    si.on_wait = keep
                    changed = True
                out.append(inst)
            if changed:
                il.clear()
                il.extend(out)
    return n_new


def build_module(split_waits=True):
    nc = bass.Bass("TRN2", target_bir_lowering=False, debug=False)

    xt_d = nc.dram_tensor("xt", [C, BL, T], BF16, kind="ExternalInput").ap()
    wq_d = nc.dram_tensor("wq", [C, C], BF16, kind="ExternalInput").ap()
    wk_d = nc.dram_tensor("wk", [C, C], BF16, kind="ExternalInput").ap()
    wv_d = nc.dram_tensor("wv", [C, C], BF16, kind="ExternalInput").ap()
    wp_d = nc.dram_tensor("wp", [C, C], BF16, kind="ExternalInput").ap()
    bp_d = nc.dram_tensor("bp", [C], F32R, kind="ExternalInput").ap()
    mask_d = nc.dram_tensor("mask", [P, 2, 2, T // 2], I16, kind="ExternalInput").ap()
    mask8_d = nc.dram_tensor("mask8", [P, 2, 2, T], FP8, kind="ExternalInput").ap()
    y_d = nc.dram_tensor("y", [BL, T, C], BF16, kind="ExternalOutput").ap()

    with tile.TileContext(nc) as tc:
        with (
            tc.tile_pool(name="consts", bufs=1) as consts,
            tc.tile_pool(name="xg", bufs=2) as xg_pool,
            tc.tile_pool(name="qt", bufs=6) as qt_pool,
            tc.tile_pool(name="kt", bufs=6) as kt_pool,
            tc.tile_pool(name="vsb", bufs=10) as v_pool,
            tc.tile_pool(name="sts", bufs=9) as sts_pool,
            tc.tile_pool(name="rbs", bufs=6) as rb_pool,
            tc.tile_pool(name="ot", bufs=4) as ot_pool,
            tc.tile_pool(name="ysb", bufs=6) as y_pool,
            tc.tile_pool(name="psbig", bufs=3, space="PSUM") as ps_big,
            tc.tile_pool(name="ps384", bufs=2, space="PSUM") as ps_384,
            tc.tile_pool(name="psorb", bufs=3, space="PSUM") as ps_orb,
        ):
            # ---- constants ----
            wq_sb = consts.tile([P, 3, C], BF16)
            wk_sb = consts.tile([P, 3, C], BF16)
            wv_sb = consts.tile([P, 3, C], BF16)
            wp_sb = consts.tile([P, 3, C], BF16)
            for w_sb, w_d in ((wq_sb, wq_d), (wk_sb, wk_d), (wv_sb, wv_d), (wp_sb, wp_d)):
                nc.sync.dma_start(w_sb[:], w_d.rearrange("(co ci) e -> ci co e", ci=P))
            # partition-replication via rank-1 matmul (ones (x) row): step-0
            # partition-broadcast DMAs produce garbage on hardware.
            ones_row = consts.tile([1, P], F32)
            nc.vector.memset(ones_row[:], 1.0)
            ones_row_r = consts.tile([1, P], F32R)
            nc.scalar.activation(
                ones_row_r[:], ones_row[:], mybir.ActivationFunctionType.Copy
            )
            bp_row = consts.tile([1, C], F32R)
            nc.sync.dma_start(bp_row[:], bp_d[None, :])
            bp_sb = consts.tile([P, C], F32)
            mask_sb = consts.tile([P, 2, 2, T // 2], I16)
            mask8_sb = consts.tile([P, 2, 2, T], FP8)
            nc.sync.dma_start(mask8_sb[:], mask8_d[:])
            nc.sync.dma_start(mask_sb[:], mask_d[:])
            bp_ps = ps_384.tile([P, C], F32, tag="ps384")
            nc.tensor.matmul(bp_ps[:], ones_row_r[0:1, :], bp_row[0:1, :], start=True, stop=True)
            nc.vector.tensor_copy(bp_sb[:], bp_ps[:])

            ones_v = consts.tile([P, 2, 64], FP8)
            nc.vector.memset(ones_v[:], 1.0)

            xt_r = xt_d.rearrange("(co ci) b t -> ci co b t", ci=P)

            def emit_tail(b, st8s, v8, v8r):
                """P@V + denominators (fp8 DR) + normalize + O-proj + y for
                batch b.  Emitted one batch late so the next batch's score
                matmuls keep the PE busy through this tail's ACT/DVE chain."""
                ot = ot_pool.tile([P, 3, T], BF16)
                for hp in range(3):
                    st8 = st8s[hp]
                    o_ps = ps_orb.tile([64, 2, T], F32, tag="psorb")
                    rb_ps = ps_orb.tile([64, 2, T], F32, tag="psorb")
                    for hidx in range(2):
                        h = 2 * hp + hidx
                        nc.tensor.matmul(
                            o_ps[:, hidx, :],
                            v8[:, :, h, :],
                            st8[:, hidx, :, :],
                            start=True, stop=False, perf_mode=DR,
                        )
                        nc.tensor.matmul(
                            o_ps[:, hidx, :],
                            v8r[:, :, h, :],
                            st8[:, hidx, :, :],
                            start=False, stop=True, perf_mode=DR,
                        )
                        nc.tensor.matmul(
                            rb_ps[:, hidx, :],
                            ones_v[:],
                            st8[:, hidx, :, :],
                            start=True, stop=True, perf_mode=DR,
                        )
                    # 1/d = exp(-ln d): both functions live in the
                    # natural_log_exp_and_others table -> no table loads
                    lnb = rb_pool.tile([64, 2, T], F32, tag="lnb")
                    nc.scalar.activation(
                        lnb[:].rearrange("p a b -> p (a b)"),
                        rb_ps[:].rearrange("p a b -> p (a b)"),
                        mybir.ActivationFunctionType.Ln,
                    )
                    rb = rb_pool.tile([64, 2, T], BF16, tag="rb")
                    nc.scalar.activation(
                        rb[:].rearrange("p a b -> p (a b)"),
                        lnb[:].rearrange("p a b -> p (a b)"),
                        mybir.ActivationFunctionType.Exp,
                        scale=-1.0,
                    )
                    # even head -> ot partitions 0:64, odd -> 64:128
                    # (DVE writes support a shifted output base partition)
                    nc.vector.tensor_mul(
                        ot[0:64, hp, :], o_ps[:, 0, :], rb[:, 0, :]
                    )
                    nc.vector.tensor_mul(
                        ot[64:128, hp, :], o_ps[:, 1, :], rb[:, 1, :]
                    )

                for tt in range(2):
                    ps = ps_384.tile([P, C], F32, tag="ps384")
                    for co in range(3):
                        nc.tensor.matmul(
                            ps[:],
                            (ot[:, co, tt * P : (tt + 1) * P]),
                            (wp_sb[:, co, :]),
                            start=(co == 0),
                            stop=(co == 2),
                        )
                    y_sb = y_pool.tile([P, C], BF16)
                    nc.vector.tensor_add(y_sb[:], ps[:], bp_sb[:])
                    nc.sync.dma_start(y_d[b, tt * P : (tt + 1) * P, :], y_sb[:])

            pending = None
            for g in range(BL // G):
                # ---- load x group [128, 3, G, T], one DMA per batch issued
                # from different engines so they land on different queues ----
                xg = xg_pool.tile([P, 3, G, T], BF16)
                for lb, eng in zip(range(G), (nc.sync, nc.scalar, nc.sync, nc.scalar)):
                    eng.dma_start(
                        xg[:, :, lb, :], xt_r[:, :, g * G + lb, :]
                    )

                # ---- Q/K projections for the group (weights stationary) ----
                qt2s, kt2s = [], []
                for w_sb, dst_list in ((wq_sb, qt2s), (wk_sb, kt2s)):
                    pool = qt_pool if w_sb is wq_sb else kt_pool
                    tg = "qtb" if w_sb is wq_sb else "ktb"
                    for bp2 in range(G // 2):
                        dst_list.append(
                            pool.tile(
                                [P, 3, 2, T], BF16, tag=tg, name=f"{tg}_{g}_{bp2}"
                            )
                        )
                    for eo in range(3):
                        for bp2 in range(G // 2):
                            ps = ps_big.tile([P, 512], F32, tag="psbig")
                            rhs = xg[:, :, 2 * bp2 : 2 * bp2 + 2, :].rearrange(
                                "p c b t -> p c (b t)"
                            )
                            for co in range(3):
                                nc.tensor.matmul(
                                    ps[:],
                                    (w_sb[:, co, eo * P : (eo + 1) * P]),
                                    (rhs[:, co, :]),
                                    start=(co == 0),
                                    stop=(co == 2),
                                )
                            dst_ap = dst_list[bp2][:, eo, :, :].rearrange(
                                "p b t -> p (b t)"
                            )
                            nc.vector.tensor_copy(dst_ap, ps[:])

                # ---- V projections for the whole group (keeps the PE busy
                # through a long ramp and frees ps_384 for the O-proj) ----
                v8s, v8rs = [], []
                for lb in range(G):
                    v8 = v_pool.tile([P, 2, H, D], FP8, tag="v8")
                    v8r = v_pool.tile([P, 2, H, D], FP8, tag="v8r")
                    for tt in range(2):
                        ps = ps_384.tile([P, C], F32, tag="ps384")
                        for co in range(3):
                            nc.tensor.matmul(
                                ps[:],
                                (xg[:, co, lb, tt * P : (tt + 1) * P]),
                                (wv_sb[:, co, :]),
                                start=(co == 0),
                                stop=(co == 2),
                            )
                        nc.vector.tensor_copy(
                            v8[:, tt, :, :].rearrange("p h d -> p (h d)"), ps[:]
                        )
                        nc.vector.tensor_sub(
                            v8r[:, tt, :, :].rearrange("p h d -> p (h d)"),
                            ps[:],
                            v8[:, tt, :, :].rearrange("p h d -> p (h d)"),
                        )
                    v8s.append(v8)
                    v8rs.append(v8r)

                for lb in range(G):
                    b = g * G + lb
                    qt = qt2s[lb // 2][:, :, lb % 2, :]
                    kt = kt2s[lb // 2][:, :, lb % 2, :]

                    # ---- scores (transposed) + exp(fp8) + mask, per pair ----
                    # causal split: the jt1 keys block only matters for the
                    # upper half of queries; (jt1, q<128) holds stale psum
                    # which the mask zeroes (bitwise AND is NaN-safe).
                    st8s = []
                    for hp in range(3):
                        st8 = sts_pool.tile([P, 2, 2, T], FP8, tag="stp")
                        for hidx in range(2):
                            h = 2 * hp + hidx
                            co, half = h // 2, h % 2
                            st_ps = ps_big.tile([P, 2, T], F32, tag="psbig")
                            nc.tensor.matmul(
                                st_ps[:, 0, :],
                                (kt[64 * half : 64 * half + 64, co, 0:P]),
                                (qt[64 * half : 64 * half + 64, co, :]),
                                start=True,
                                stop=True,
                            )
                            nc.tensor.matmul(
                                st_ps[:, 1, P:T],
                                (kt[64 * half : 64 * half + 64, co, P:T]),
                                (qt[64 * half : 64 * half + 64, co, P:T]),
                                start=True,
                                stop=True,
                            )
                            nc.scalar.activation(
                                st8[:, hidx, :, :],
                                st_ps[:],
                                mybir.ActivationFunctionType.Exp,
                            )
                        # one int16-view bitwise AND applies the triangle to
                        # the diagonal blocks AND zeros the (jt1, q<128) block;
                        # the middle pair rides the otherwise-idle gpsimd as a
                        # plain fp8 multiply (no bitwise ops on Pool; stale
                        # values there are old finite exps, so x*0 == 0)
                        if hp == 1:
                            nc.gpsimd.tensor_mul(
                                st8[:].rearrange("p h a b -> p (h a b)"),
                                st8[:].rearrange("p h a b -> p (h a b)"),
                                mask8_sb[:].rearrange("p h a b -> p (h a b)"),
                            )
                        else:
                            st_i16 = st8[:].rearrange("p h a b -> p (h a b)").bitcast(I16)
                            nc.vector.tensor_tensor(
                                st_i16,
                                st_i16,
                                mask_sb[:].rearrange("p h a b -> p (h a b)"),
                                mybir.AluOpType.bitwise_and,
                            )
                        st8s.append(st8)

                    if pending is not None:
                        emit_tail(*pending)
                    pending = (b, st8s, v8s[lb], v8rs[lb])

            emit_tail(*pending)

    if split_waits:
        _split_drain_waits(nc)
    return nc


_NC = None


def _get_nc():
    global _NC
    if _NC is None:
        _NC = build_module()
    return _NC


def make_mask():
    # [128, 2(head), 2(jt), 128] int16 byte-pair visibility masks over the
    # fp8 st tile: key jt*128+p visible to query q when jt*128+p <= q.
    # Little-endian byte pair (q=2m, q=2m+1) -> 0xFF00 keeps q=2m+1 only.
    j = np.arange(P)[:, None]
    q = np.arange(T)[None, :]
    keep = np.empty((P, 2, T), bool)
    keep[:, 0, :] = j <= q          # keys 0:128
    keep[:, 1, :] = j + P <= q      # keys 128:256
    lo = keep[:, :, 0::2].astype(np.uint16) * 0x00FF
    hi = keep[:, :, 1::2].astype(np.uint16) * 0xFF00
    m = (lo | hi).view(np.int16)    # [P, 2(jt), 128]
    return np.broadcast_to(m[:, None, :, :], (P, 2, 2, T // 2)).copy()


def make_mask8():
    # same visibility as make_mask but as fp8 0/1 for the gpsimd multiply
    import ml_dtypes

    j = np.arange(P)[:, None]
    q = np.arange(T)[None, :]
    m = np.empty((P, 2, T), np.float32)
    m[:, 0, :] = j <= q
    m[:, 1, :] = j + P <= q
    m8 = m.astype(ml_dtypes.float8_e4m3)
    return np.broadcast_to(m8[:, None, :, :], (P, 2, 2, T)).copy()


def prepare_in_maps(x, Wk, Wq, Wv, Wp, bp):
    import ml_dtypes

    bf16 = ml_dtypes.bfloat16
    xt = np.ascontiguousarray(
        np.asarray(x, dtype=np.float32).transpose(2, 0, 1).astype(bf16)
    )
    # 1/sqrt(D) folded into Wq (exact exponent shift in bf16)
    wq = np.ascontiguousarray((np.asarray(Wq, dtype=np.float32).T * 0.125).astype(bf16))
    wk = np.ascontiguousarray(np.asarray(Wk, dtype=np.float32).T.astype(bf16))
    wv = np.ascontiguousarray(np.asarray(Wv, dtype=np.float32).T.astype(bf16))
    wp = np.ascontiguousarray(np.asarray(Wp, dtype=np.float32).T.astype(bf16))
    bp = np.asarray(bp, dtype=np.float32)
    mask = make_mask()
    mask8 = make_mask8()
    in_maps = []
    for c in range(NCORES):
        in_maps.append(
            {
                "xt": np.ascontiguousarray(xt[:, c * BL : (c + 1) * BL, :]),
                "wq": wq,
                "wk": wk,
                "wv": wv,
                "wp": wp,
                "bp": bp,
                "mask": mask,
                "mask8": mask8,
            }
        )
    return in_maps


def kernel(x, Wk, Wq, Wv, Wp, bp):
    nc = _get_nc()
    in_maps = prepare_in_maps(x, Wk, Wq, Wv, Wp, bp)
    res = run_bass_kernel_spmd(nc, in_maps, list(range(NCORES)))
    return np.concatenate(
        [np.asarray(r["y"], dtype=np.float32) for r in res.results], axis=0
    )
